# revision 2
# baseline (speedup 1.0000x reference)
"""CrossCoderDecoder forward on 8 trn2 NeuronCores.

x[b,l,d] = sum_f f[b,f] * weight[l,f,d] + bias[l,d]
B=32, L=2, F=65536, D=768, fp32.

Sharding: the F (dict) axis is split 8 ways (8192 features per core).
Each core computes its partial [L, 2*B, D] sums; the host sums the 8
partials (and the fh/fl half-pair, see below) and adds the bias (the
"all-reduce" of the sharding hint, done host-side since the output is
tiny).

Precision/perf scheme: the kernel is HBM-bound on streaming the weight
(L*FS*D elements/core, each used once), so bytes/element is the whole
game. The weight is stored as SINGLE bf16 (2 B/elem, vs 4 B/elem for
fp32 or a hi/lo bf16 pair): one streaming pass on the PE at 1 col/cyc.
f stays fp32-grade via a hi/lo bf16 split packed side by side in the
stationary operand ([128, 64] lhsT = [fh | fl]), so

    psum[0:32]  += fh . w
    psum[32:64] += fl . w

and the host adds the two halves. Total error is dominated by the w
bf16 rounding: ~2.5e-3 max-rel (vs the 2e-2 gate).

Weight DMA layout: per (l, chunk of CHUNK_ROWS k-rows) one dma_start
moves a contiguous [P, KO, D] block (~786 KB, 6 KB/partition line)
into SBUF. The host pre-packs the weights into exactly that image and
pre-permutes f into fhl[p, j, 64] with the matching k order
(k = ch*CHUNK_ROWS + p*KO + o), so the contraction stays consistent.

The l loop is sequential (all l=0 chunks, then l=1) so l=0's PSUM
drain + output DMA overlap l=1's compute; output DMAs ride the gpsimd
SWDGE path so they never head-of-line-block the two HWDGE weight
rings (sync/scalar, alternated per chunk).
"""

import numpy as np
import ml_dtypes

import concourse.bass as bass
import concourse.tile as tile
from concourse import bacc, mybir
from concourse import bass_utils

B, L, F, D = 32, 2, 65536, 768
NCORES = 8
FS = F // NCORES          # 8192 features per core
P = 128
CHUNK_ROWS = 512          # k-rows per weight DMA
CH = FS // CHUNK_ROWS     # chunks per l
KO = CHUNK_ROWS // P      # k-subtiles per chunk
W_BUFS = 16               # weight tile double-buffering depth
NSPLITS = ((0, 512), (512, 768))  # PSUM-bank splits of D

_F32 = mybir.dt.float32
_BF16 = mybir.dt.bfloat16
_BF16_NP = ml_dtypes.bfloat16

_cache = {}


def set_tiling(chunk_rows: int, w_bufs: int | None = None):
    """Adjust chunking (for tuning sweeps); drops the cached program."""
    global CHUNK_ROWS, CH, KO, W_BUFS
    CHUNK_ROWS = chunk_rows
    CH = FS // CHUNK_ROWS
    KO = CHUNK_ROWS // P
    if w_bufs is not None:
        W_BUFS = w_bufs
    _cache.clear()


def _build():
    """Build + schedule the (per-core identical) Bass program once."""
    nc = bacc.Bacc("TRN2", target_bir_lowering=False, debug=False)

    fhl = nc.dram_tensor("fhl", [P, CH * KO, 2 * B], _BF16, kind="ExternalInput").ap()
    w = nc.dram_tensor("w", [L, CH, P, KO, D], _BF16, kind="ExternalInput").ap()
    out = nc.dram_tensor("out", [L, 2 * B, D], _F32, kind="ExternalOutput").ap()

    with tile.TileContext(nc) as tc:
        with (
            tc.tile_pool(name="fpool", bufs=1) as fpool,
            tc.tile_pool(name="wpool", bufs=W_BUFS) as wpool,
            tc.tile_pool(name="opool", bufs=2) as opool,
            tc.tile_pool(name="psum", bufs=1, space="PSUM") as psum,
        ):
            # fhl rides the ACT HWDGE ring so it overlaps the first w
            # chunks (the SP ring is FIFO per issuing engine).
            f_sb = fpool.tile([P, CH * KO, 2 * B], _BF16)
            nc.scalar.dma_start(f_sb[:], fhl[:])

            ps = [
                [
                    psum.tile([2 * B, n1 - n0], _F32, name=f"ps_{l}_{i}")
                    for i, (n0, n1) in enumerate(NSPLITS)
                ]
                for l in range(L)
            ]
            for l in range(L):
                for ch in range(CH):
                    wt = wpool.tile([P, KO, D], _BF16)
                    dma_eng = nc.sync if ch % 2 == 0 else nc.scalar
                    dma_eng.dma_start(wt[:], w[l, ch])
                    for o in range(KO):
                        j = ch * KO + o
                        for i, (n0, n1) in enumerate(NSPLITS):
                            nc.tensor.matmul(
                                ps[l][i][:],
                                f_sb[:, j, :],
                                wt[:, o, n0:n1],
                                start=(j == 0),
                                stop=(j == CH * KO - 1),
                            )
                # Drain l's PSUM while the next l's chunks stream.
                out_sb = opool.tile([2 * B, D], _F32)
                for i, (n0, n1) in enumerate(NSPLITS):
                    nc.vector.tensor_copy(out=out_sb[:, n0:n1], in_=ps[l][i][:])
                nc.gpsimd.dma_start(out[l], out_sb[:])

    nc.compile()
    return nc


def _split_hl(x: np.ndarray):
    """fp32 -> (hi, lo) bf16 pair with x ~= hi + lo."""
    hi = x.astype(_BF16_NP)
    lo = (x - hi.astype(np.float32)).astype(_BF16_NP)
    return hi, lo


def _prep_f(f_core: np.ndarray) -> np.ndarray:
    """f_core [B, FS] -> fhl [P, CH*KO, 2*B] bf16 matching the kernel's
    k order (k = ch*CHUNK_ROWS + p*KO + o at fhl[p, ch*KO + o]); the
    last axis holds fh[b] in [0, B) and fl[b] in [B, 2B)."""
    hi, lo = _split_hl(f_core)
    ft = np.concatenate([hi.T, lo.T], axis=1)          # [FS, 2B]
    ft = ft.reshape(CH, P, KO, 2 * B).transpose(1, 0, 2, 3)
    return np.ascontiguousarray(ft.reshape(P, CH * KO, 2 * B))


def _prep_w(w_core: np.ndarray) -> np.ndarray:
    """w_core [L, FS, D] -> [L, CH, P, KO, D] bf16 (exact SBUF image)."""
    hi = w_core.astype(_BF16_NP)
    return np.ascontiguousarray(hi.reshape(L, CH, P, KO, D))


def kernel(f: np.ndarray, weight: np.ndarray, bias: np.ndarray) -> np.ndarray:
    f = np.asarray(f, dtype=np.float32)
    weight = np.asarray(weight, dtype=np.float32)
    bias = np.asarray(bias, dtype=np.float32)

    if "nc" not in _cache:
        _cache["nc"] = _build()
    nc = _cache["nc"]

    in_maps = []
    for c in range(NCORES):
        sl = slice(c * FS, (c + 1) * FS)
        in_maps.append(
            {
                "fhl": _prep_f(f[:, sl]),
                "w": _prep_w(weight[:, sl, :]),
            }
        )

    res = bass_utils.run_bass_kernel_spmd(nc, in_maps, core_ids=list(range(NCORES)))
    partial = np.stack([r["out"] for r in res.results])  # [NCORES, L, 2B, D]
    total = partial.sum(axis=0)                          # [L, 2B, D]
    total = total[:, :B, :] + total[:, B:, :]            # fh-half + fl-half
    x = total.transpose(1, 0, 2) + bias[None, :, :]      # [B, L, D]
    return x.astype(np.float32)


# revision 6
# speedup vs baseline: 1.0542x; 1.0542x over previous
"""CrossCoderDecoder forward on 8 trn2 NeuronCores.

x[b,l,d] = sum_f f[b,f] * weight[l,f,d] + bias[l,d]
B=32, L=2, F=65536, D=768, fp32.

Sharding: the F (dict) axis is split 8 ways (8192 features per core).
Each core computes its partial [L, B, D] sums; the host sums the 8
partials and adds the bias (the "all-reduce" of the sharding hint,
done host-side since the output is tiny).

Precision/perf scheme: the kernel is HBM-bound on streaming the weight
(L*FS*D elements/core, each used once), so bytes/element is the whole
game. Both f and weight are cast to SINGLE bf16 (2 B/elem vs fp32's
4): one streaming pass on the PE at 1 col/cyc, fp32 PSUM accumulate.
Total error ~2e-3 max-rel vs the 2e-2 gate.

Weight DMA layout: per (chunk of R k-rows, l) one dma_start moves a
contiguous [P, R/P, D] block into SBUF. Chunk sizes taper
(3×2048, 1024, 512, 256, 256 rows = 3.15 MB bulk transfers for DMA
efficiency, small trailing chunks so the end-of-stream completion
latency + final matmul burst expose as little as possible). Both l
share one chunk schedule so the (host pre-permuted) f image
fhl[p, j, b] (k = kofs(chunk) + p*(R/P) + o at j = jofs(chunk)+o)
serves both l. Chunks interleave l (c0l0, c0l1, c1l0, ...) with all
four PSUM accumulators open across the whole kernel; the weight DMAs
alternate between the two HWDGE rings (sync/scalar). The final PSUM
drains split across the vector+scalar engines and the two output DMAs
ride the by-then-empty sync/scalar rings.
"""

import contextlib

import numpy as np
import ml_dtypes

import concourse.bass as bass
import concourse.tile as tile
from concourse import bacc, mybir
from concourse import bass_utils

B, L, F, D = 32, 2, 65536, 768
NCORES = 8
FS = F // NCORES          # 8192 features per core
P = 128
CHUNKS = (2048, 2048, 2048, 1024, 512, 256, 256)  # k-rows per weight DMA
W_BUFS = {16: 4, 8: 2, 4: 2, 2: 4}                # per-size-class pool depth
NSPLITS = ((0, 512), (512, 768))  # PSUM-bank splits of D

assert sum(CHUNKS) == FS
_KOS = [r // P for r in CHUNKS]                   # k-subtiles per chunk
_NJ = sum(_KOS)                                   # 64 subtiles per l
_CLASSES = sorted(set(_KOS), reverse=True)        # distinct chunk sizes

_F32 = mybir.dt.float32
_BF16 = mybir.dt.bfloat16
_BF16_NP = ml_dtypes.bfloat16

_cache = {}


def set_chunks(chunks: tuple, w_bufs: dict | None = None):
    """Adjust chunking (for tuning sweeps); drops the cached program."""
    global CHUNKS, _KOS, _NJ, _CLASSES
    CHUNKS = tuple(chunks)
    assert sum(CHUNKS) == FS
    _KOS = [r // P for r in CHUNKS]
    _NJ = sum(_KOS)
    _CLASSES = sorted(set(_KOS), reverse=True)
    if w_bufs is not None:
        W_BUFS.update(w_bufs)
    _cache.clear()


def _build():
    """Build + schedule the (per-core identical) Bass program once."""
    nc = bacc.Bacc("TRN2", target_bir_lowering=False, debug=False)

    fhl = nc.dram_tensor("fhl", [P, _NJ, B], _BF16, kind="ExternalInput").ap()
    wdram = {
        ko: nc.dram_tensor(
            f"w{ko}", [_KOS.count(ko), L, P, ko, D], _BF16, kind="ExternalInput"
        ).ap()
        for ko in _CLASSES
    }
    out = nc.dram_tensor("out", [L, B, D], _F32, kind="ExternalOutput").ap()

    with tile.TileContext(nc) as tc:
        with contextlib.ExitStack() as stack:
            fpool = stack.enter_context(tc.tile_pool(name="fpool", bufs=1))
            opool = stack.enter_context(tc.tile_pool(name="opool", bufs=2))
            psum = stack.enter_context(
                tc.tile_pool(name="psum", bufs=1, space="PSUM")
            )
            wpools = {
                ko: stack.enter_context(
                    tc.tile_pool(name=f"wpool{ko}", bufs=W_BUFS[ko])
                )
                for ko in _CLASSES
            }

            # fhl rides the ACT HWDGE ring so it overlaps the first w
            # chunk (the SP ring is FIFO per issuing engine).
            f_sb = fpool.tile([P, _NJ, B], _BF16)
            nc.scalar.dma_start(f_sb[:], fhl[:])

            ps = [
                [
                    psum.tile([B, n1 - n0], _F32, name=f"ps_{l}_{i}")
                    for i, (n0, n1) in enumerate(NSPLITS)
                ]
                for l in range(L)
            ]
            dma_i = 0
            jofs = 0
            cls_idx = {ko: 0 for ko in _CLASSES}
            for ci, r in enumerate(CHUNKS):
                ko = r // P
                for l in range(L):
                    wt = wpools[ko].tile([P, ko, D], _BF16)
                    dma_eng = nc.sync if dma_i % 2 == 0 else nc.scalar
                    dma_eng.dma_start(wt[:], wdram[ko][cls_idx[ko], l])
                    dma_i += 1
                    for o in range(ko):
                        j = jofs + o
                        for i, (n0, n1) in enumerate(NSPLITS):
                            nc.tensor.matmul(
                                ps[l][i][:],
                                f_sb[:, j, :],
                                wt[:, o, n0:n1],
                                start=(j == 0),
                                stop=(j == _NJ - 1),
                            )
                cls_idx[ko] += 1
                jofs += ko
            # Drain: copies split across vector+scalar, out DMAs on the
            # two HWDGE rings (empty once the last w chunk is queued).
            for l in range(L):
                out_sb = opool.tile([B, D], _F32)
                nc.vector.tensor_copy(out=out_sb[:, 0:512], in_=ps[l][0][:])
                nc.scalar.copy(out=out_sb[:, 512:768], in_=ps[l][1][:])
                (nc.sync if l == 0 else nc.scalar).dma_start(out[l], out_sb[:])

    nc.compile()
    return nc


def _prep_core(f_core: np.ndarray, w_core: np.ndarray) -> dict:
    """Build the per-core input map.

    f_core [B, FS] fp32 -> fhl [P, NJ, B] bf16 with
    fhl[p, jofs+o, b] = f[b, kofs + p*ko + o] per chunk.
    w_core [L, FS, D] fp32 -> one [cnt, L, P, ko, D] bf16 image per
    chunk-size class, matching the kernel's DMA order.
    """
    fh = f_core.astype(_BF16_NP)
    wh = w_core.astype(_BF16_NP)
    fhl = np.empty((P, _NJ, B), dtype=_BF16_NP)
    wimgs = {ko: [] for ko in _CLASSES}
    kofs = 0
    jofs = 0
    for r in CHUNKS:
        ko = r // P
        # k = kofs + p*ko + o  (C-order reshape)
        fhl[:, jofs : jofs + ko, :] = (
            fh[:, kofs : kofs + r].T.reshape(P, ko, B)
        )
        wimgs[ko].append(wh[:, kofs : kofs + r, :].reshape(L, P, ko, D))
        kofs += r
        jofs += ko
    in_map = {"fhl": np.ascontiguousarray(fhl)}
    for ko in _CLASSES:
        in_map[f"w{ko}"] = np.ascontiguousarray(np.stack(wimgs[ko]))
    return in_map


def kernel(f: np.ndarray, weight: np.ndarray, bias: np.ndarray) -> np.ndarray:
    f = np.asarray(f, dtype=np.float32)
    weight = np.asarray(weight, dtype=np.float32)
    bias = np.asarray(bias, dtype=np.float32)

    if "nc" not in _cache:
        _cache["nc"] = _build()
    nc = _cache["nc"]

    in_maps = []
    for c in range(NCORES):
        sl = slice(c * FS, (c + 1) * FS)
        in_maps.append(_prep_core(f[:, sl], weight[:, sl, :]))

    res = bass_utils.run_bass_kernel_spmd(nc, in_maps, core_ids=list(range(NCORES)))
    partial = np.stack([r["out"] for r in res.results])  # [NCORES, L, B, D]
    total = partial.sum(axis=0)                          # [L, B, D]
    x = total.transpose(1, 0, 2) + bias[None, :, :]      # [B, L, D]
    return x.astype(np.float32)


# revision 7
# speedup vs baseline: 1.0555x; 1.0012x over previous
"""CrossCoderDecoder forward on 8 trn2 NeuronCores.

x[b,l,d] = sum_f f[b,f] * weight[l,f,d] + bias[l,d]
B=32, L=2, F=65536, D=768, fp32.

Sharding: the F (dict) axis is split 8 ways (8192 features per core).
Each core computes its partial [L, B, D] sums; the host sums the 8
partials and adds the bias (the "all-reduce" of the sharding hint,
done host-side since the output is tiny).

Precision/perf scheme: the kernel is HBM-bound on streaming the weight
(L*FS*D elements/core, each used once), so bytes/element is the whole
game. Both f and weight are cast to SINGLE bf16 (2 B/elem vs fp32's
4): one streaming pass on the PE at 1 col/cyc, fp32 PSUM accumulate.
Total error ~2e-3 max-rel vs the 2e-2 gate.

Weight DMA layout: per chunk of R k-rows ONE dma_start moves a
contiguous [P, R/P, L, D] block (both l interleaved per k-row) into
SBUF. Chunk sizes taper (7x1024, 512, 256, 128, 128 rows: 3.1 MB bulk
transfers for DMA efficiency, small trailing chunks so the
end-of-stream completion latency + final matmul burst expose as
little as possible). All four PSUM accumulators stay open across the
whole kernel; the weight DMAs alternate between the two HWDGE rings
(sync/scalar). The final PSUM drains split across the vector+scalar
engines and the two output DMAs ride the by-then-empty sync/scalar
rings. A single SBUF tile pool with per-tag buffer groups keeps the
Tile semaphore/barrier overhead (prologue+epilogue) down.

Host-side prep packs the weight into the exact SBUF images
(k = kofs(chunk) + p*(R/P) + o at image[p, o]) and permutes f into
fhl[p, j, b] with the matching k order, so the contraction stays
consistent.
"""

import contextlib

import numpy as np
import ml_dtypes

import concourse.bass as bass
import concourse.tile as tile
from concourse import bacc, mybir
from concourse import bass_utils

B, L, F, D = 32, 2, 65536, 768
NCORES = 8
FS = F // NCORES          # 8192 features per core
P = 128
CHUNKS = (1024, 1024, 1024, 1024, 1024, 1024, 1024, 512, 256, 128, 128)
W_BUFS = {8: 4, 4: 1, 2: 1, 1: 2}                 # per-size-class bufs
NSPLITS = ((0, 512), (512, 768))  # PSUM-bank splits of D

assert sum(CHUNKS) == FS
_KOS = [r // P for r in CHUNKS]                   # k-subtiles per chunk
_NJ = sum(_KOS)                                   # 64 subtiles
_CLASSES = sorted(set(_KOS), reverse=True)        # distinct chunk sizes

_F32 = mybir.dt.float32
_BF16 = mybir.dt.bfloat16
_BF16_NP = ml_dtypes.bfloat16

_cache = {}


def set_chunks(chunks: tuple, w_bufs: dict | None = None):
    """Adjust chunking (for tuning sweeps); drops the cached program."""
    global CHUNKS, _KOS, _NJ, _CLASSES
    CHUNKS = tuple(chunks)
    assert sum(CHUNKS) == FS
    _KOS = [r // P for r in CHUNKS]
    _NJ = sum(_KOS)
    _CLASSES = sorted(set(_KOS), reverse=True)
    if w_bufs is not None:
        W_BUFS.update(w_bufs)
    _cache.clear()


def _build():
    """Build + schedule the (per-core identical) Bass program once."""
    nc = bacc.Bacc("TRN2", target_bir_lowering=False, debug=False)

    fhl = nc.dram_tensor("fhl", [P, _NJ, B], _BF16, kind="ExternalInput").ap()
    wdram = {
        ko: nc.dram_tensor(
            f"w{ko}", [_KOS.count(ko), P, ko, L, D], _BF16, kind="ExternalInput"
        ).ap()
        for ko in _CLASSES
    }
    out = nc.dram_tensor("out", [L, B, D], _F32, kind="ExternalOutput").ap()

    with tile.TileContext(nc) as tc:
        with (
            tc.tile_pool(name="sb", bufs=1) as sb,
            tc.tile_pool(name="psum", bufs=1, space="PSUM") as psum,
        ):
            # fhl rides the ACT HWDGE ring so it overlaps the first w
            # chunk (the SP ring is FIFO per issuing engine).
            f_sb = sb.tile([P, _NJ, B], _BF16, tag="f", bufs=1, name="f_sb")
            nc.scalar.dma_start(f_sb[:], fhl[:])

            ps = [
                [
                    psum.tile([B, n1 - n0], _F32, name=f"ps_{l}_{i}")
                    for i, (n0, n1) in enumerate(NSPLITS)
                ]
                for l in range(L)
            ]
            jofs = 0
            cls_idx = {ko: 0 for ko in _CLASSES}
            for ci, r in enumerate(CHUNKS):
                ko = r // P
                wt = sb.tile(
                    [P, ko, L, D], _BF16, tag=f"w{ko}", bufs=W_BUFS[ko],
                    name=f"wt{ci}",
                )
                dma_eng = nc.sync if ci % 2 == 0 else nc.scalar
                dma_eng.dma_start(wt[:], wdram[ko][cls_idx[ko]])
                for o in range(ko):
                    j = jofs + o
                    for l in range(L):
                        for i, (n0, n1) in enumerate(NSPLITS):
                            nc.tensor.matmul(
                                ps[l][i][:],
                                f_sb[:, j, :],
                                wt[:, o, l, n0:n1],
                                start=(j == 0),
                                stop=(j == _NJ - 1),
                            )
                cls_idx[ko] += 1
                jofs += ko
            # Drain: copies split across vector+scalar, out DMAs on the
            # two HWDGE rings (empty once the last w chunk is queued).
            for l in range(L):
                out_sb = sb.tile([B, D], _F32, tag="o", bufs=2, name=f"o{l}")
                nc.vector.tensor_copy(out=out_sb[:, 0:512], in_=ps[l][0][:])
                nc.scalar.copy(out=out_sb[:, 512:768], in_=ps[l][1][:])
                (nc.sync if l == 0 else nc.scalar).dma_start(out[l], out_sb[:])

    nc.compile()
    return nc


def _prep_core(f_core: np.ndarray, w_core: np.ndarray) -> dict:
    """Build the per-core input map.

    f_core [B, FS] fp32 -> fhl [P, NJ, B] bf16 with
    fhl[p, jofs+o, b] = f[b, kofs + p*ko + o] per chunk.
    w_core [L, FS, D] fp32 -> one [cnt, P, ko, L, D] bf16 image per
    chunk-size class, matching the kernel's DMA order.
    """
    fh = f_core.astype(_BF16_NP)
    wh = w_core.astype(_BF16_NP)          # [L, FS, D]
    whT = np.ascontiguousarray(wh.transpose(1, 0, 2))  # [FS, L, D]
    fhl = np.empty((P, _NJ, B), dtype=_BF16_NP)
    wimgs = {ko: [] for ko in _CLASSES}
    kofs = 0
    jofs = 0
    for r in CHUNKS:
        ko = r // P
        # k = kofs + p*ko + o  (C-order reshape)
        fhl[:, jofs : jofs + ko, :] = (
            fh[:, kofs : kofs + r].T.reshape(P, ko, B)
        )
        wimgs[ko].append(whT[kofs : kofs + r].reshape(P, ko, L, D))
        kofs += r
        jofs += ko
    in_map = {"fhl": np.ascontiguousarray(fhl)}
    for ko in _CLASSES:
        in_map[f"w{ko}"] = np.ascontiguousarray(np.stack(wimgs[ko]))
    return in_map


def kernel(f: np.ndarray, weight: np.ndarray, bias: np.ndarray) -> np.ndarray:
    f = np.asarray(f, dtype=np.float32)
    weight = np.asarray(weight, dtype=np.float32)
    bias = np.asarray(bias, dtype=np.float32)

    if "nc" not in _cache:
        _cache["nc"] = _build()
    nc = _cache["nc"]

    in_maps = []
    for c in range(NCORES):
        sl = slice(c * FS, (c + 1) * FS)
        in_maps.append(_prep_core(f[:, sl], weight[:, sl, :]))

    res = bass_utils.run_bass_kernel_spmd(nc, in_maps, core_ids=list(range(NCORES)))
    partial = np.stack([r["out"] for r in res.results])  # [NCORES, L, B, D]
    total = partial.sum(axis=0)                          # [L, B, D]
    x = total.transpose(1, 0, 2) + bias[None, :, :]      # [B, L, D]
    return x.astype(np.float32)
